# revision 2
# baseline (speedup 1.0000x reference)
"""nn_CollectConcat: bilinear deformable sampling, Trainium2 Bass kernel (v3).

x[4,720,128,128]: 9 points x 80-channel blocks, each bilinearly sampled at
per-pixel locations from location[4,18,128,128], plus per-channel bias.

Sharding (pure data parallel, 8 cores): core c -> batch b=c//2, side c%2.
Side 0 owns points [0..3] + first half of point 4's positions; side 1 owns
points [5..8] + second half of point 4 (its location rows are rotated by
8192 on the host so the program always computes "positions 0..8191").

Pipeline per point: fp16 pair table in DRAM (row i = [x(i)|x(i+128)], so one
640B descriptor = all 4 bilinear corners), SWDGE indirect gathers capped at
DESC_COLS*128 descriptors/op (descriptor-ring safety), fused DVE combine
(quad fp16 weights, one broadcast mult + 2 adds), PE transpose + ACT bias.
"""
import sys
sys.path.insert(0, '/opt/trn_rl_repo')
import numpy as np
import concourse.bass as bass
import concourse.bacc as bacc
import concourse.mybir as mybir
import concourse.tile as tile
import concourse.tile_sem_assignment as tsa
tsa.NUM_HWDGE_SEMS = 7  # 7 HW + 1 SW DMA sem domains: tail drain wait limit
from concourse.masks import make_identity

F32 = mybir.dt.float32
F16 = mybir.dt.float16
I32 = mybir.dt.int32
Alu = mybir.AluOpType
Act = mybir.ActivationFunctionType

B, C, H, W = 4, 720, 128, 128
P, OFF = 9, 80
HW = H * W                      # 16384
NPTS = 5                        # point-blocks per core
GB = 8                          # position-groups (of 128) per batch
POSB = GB * 128                 # positions per batch (1024)
QC = 4096                       # x-load quarter width
JQ = QC // 128                  # j iterations per quarter (32)
DESC_COLS = 1                   # idx columns (128 desc each) per indirect op
HALF_SPLIT = True               # point index 4: only positions 0..8191

_nc_cache = {}


def build_nc(npts=NPTS, desc_cols=None, half_split=None):
    # desc_cols: int, or list of per-point values (diagnostic builds)
    desc_cols = DESC_COLS if desc_cols is None else desc_cols
    if not isinstance(desc_cols, (list, tuple)):
        desc_cols = [desc_cols] * npts
    desc_cols = tuple(desc_cols)
    half_split = HALF_SPLIT if half_split is None else half_split
    key = (npts, desc_cols, half_split)
    if key in _nc_cache:
        return _nc_cache[key]
    nc = bacc.Bacc("TRN2")
    xblk = nc.declare_dram_parameter("xblk", [npts, OFF, HW], F32, isOutput=False)
    locp = nc.declare_dram_parameter("locp", [npts, 2, HW], F32, isOutput=False)
    biasb = nc.declare_dram_parameter("biasb", [npts, OFF], F32, isOutput=False)
    outb = nc.declare_dram_parameter("outb", [npts, OFF, HW], F32, isOutput=True)
    tabs = [nc.dram_tensor(f"t2_{p}", [HW, 160], F16) for p in range(npts)]

    with tile.TileContext(nc) as tc:
        with tc.tile_pool(name="sbuf", bufs=2) as sb, \
             tc.tile_pool(name="cst", bufs=1) as cst, \
             tc.tile_pool(name="psc", bufs=1, space="PSUM") as ppc, \
             tc.tile_pool(name="pst", bufs=2, space="PSUM") as ppt, \
             tc.tile_pool(name="pso", bufs=2, space="PSUM") as ppo:
            ident = cst.tile([128, 128], F32)
            make_identity(nc, ident[:])
            ident16 = cst.tile([128, 128], F16, tag="id16", name="id16")
            nc.vector.tensor_copy(ident16[:], ident[:])
            warm = ppc.tile([128, 128], F32, tag="warm")
            nc.tensor.transpose(warm[:], ident[:], ident[:])  # PE sees ident

            bias_t = [cst.tile([OFF, 1], F32, tag=f"bias{p}", name=f"bias{p}") for p in range(npts)]
            for p in range(npts):
                nc.sync.dma_start(bias_t[p][:],
                                  bass.AP(biasb, p * OFF, [[1, OFF], [1, 1]]))

            for p in range(npts):
                # ---- stage T: build fp16 pair table t2[p][i] = [x(i)|x(i+128)]
                for q in range(4):
                    img = sb.tile([OFF, QC + 128], F32, tag="img")
                    base = q * QC
                    ncols = QC + 128 if q < 3 else QC
                    nc.sync.dma_start(img[:, 0:ncols],
                                      xblk[p, :, base:base + ncols])
                    if q == 3:
                        nc.vector.memset(img[:, QC:QC + 128], 0.0)
                    outh = sb.tile([128, JQ * 160], F16, tag="outh")
                    istr = img[:].ap[0][0]
                    j = 0
                    while j < JQ:
                        jn = min(3, JQ - j)
                        ps = ppt.tile([128, 480], F32, tag="tps")
                        for jj in range(jn):
                            in0 = bass.AP(img.tensor, j + jj, [[istr, OFF], [JQ, 128]])
                            in1 = bass.AP(img.tensor, j + jj + 128, [[istr, OFF], [JQ, 128]])
                            nc.tensor.transpose(ps[:, jj * 160:jj * 160 + 80],
                                                in0, ident[0:OFF, 0:OFF])
                            nc.tensor.transpose(ps[:, jj * 160 + 80:jj * 160 + 160],
                                                in1, ident[0:OFF, 0:OFF])
                        nc.scalar.activation(outh[:, j * 160:(j + jn) * 160],
                                             ps[:, 0:jn * 160], Act.Copy)
                        j += jn
                    # partition k -> rows base+JQ*k .. +JQ  (10KB runs)
                    nc.sync.dma_start(
                        bass.AP(tabs[p], base * 160, [[JQ * 160, 128], [1, JQ * 160]]),
                        outh[:])

                # ---- stage L: locations -> idx + quad weights ----
                locy_r = sb.tile([128, 128], F32, tag="locyr")
                locx_r = sb.tile([128, 128], F32, tag="locxr")
                nc.sync.dma_start(locy_r[:], bass.AP(locp, (2 * p) * HW, [[128, 128], [1, 128]]))
                nc.sync.dma_start(locx_r[:], bass.AP(locp, (2 * p + 1) * HW, [[128, 128], [1, 128]]))
                psy = ppc.tile([128, 128], F32, tag="locps")
                nc.tensor.transpose(psy[:], locy_r[:], ident[:])
                yq = sb.tile([128, 128], F32, tag="yq")
                nc.scalar.activation(yq[:], psy[:], Act.Copy)
                psx = ppc.tile([128, 128], F32, tag="locps")
                nc.tensor.transpose(psx[:], locx_r[:], ident[:])
                xq = sb.tile([128, 128], F32, tag="xq")
                nc.scalar.activation(xq[:], psx[:], Act.Copy)

                # floor via int roundtrip + is_gt correction (mode-agnostic)
                def floorf(q_, tag):
                    qi = sb.tile([128, 128], I32, tag=tag + "i")
                    nc.vector.tensor_copy(qi[:], q_[:])
                    qf = sb.tile([128, 128], F32, tag=tag + "f")
                    nc.vector.tensor_copy(qf[:], qi[:])
                    gt = sb.tile([128, 128], F32, tag=tag + "g")
                    nc.vector.tensor_tensor(out=gt[:], in0=qf[:], in1=q_[:], op=Alu.is_gt)
                    nc.vector.tensor_sub(qf[:], qf[:], gt[:])
                    d = sb.tile([128, 128], F32, tag=tag + "d")
                    nc.vector.tensor_sub(d[:], q_[:], qf[:])
                    return qf, d
                y0f, dy = floorf(yq, "y")
                x0f, dx = floorf(xq, "x")

                idxf = sb.tile([128, 128], F32, tag="idxf")
                nc.vector.scalar_tensor_tensor(out=idxf[:], in0=y0f[:], scalar=128.0,
                                               in1=x0f[:], op0=Alu.mult, op1=Alu.add)
                idx = sb.tile([128, 128], I32, tag="idx")
                nc.vector.tensor_copy(idx[:], idxf[:])

                omy = sb.tile([128, 128], F32, tag="omy")
                nc.vector.tensor_scalar(out=omy[:], in0=dy[:], scalar1=-1.0,
                                        scalar2=1.0, op0=Alu.mult, op1=Alu.add)
                omx = sb.tile([128, 128], F32, tag="omx")
                nc.vector.tensor_scalar(out=omx[:], in0=dx[:], scalar1=-1.0,
                                        scalar2=1.0, op0=Alu.mult, op1=Alu.add)
                # quad weights, fp16, interleaved: wq[c, pp, k]
                # pair-table corner order: [0:80]=(y0,x0) [80:160]=(y1,x0)
                #                          [160:240]=(y0,x1) [240:320]=(y1,x1)
                wq = sb.tile([128, 128, 4], F16, tag="wq")
                nc.vector.tensor_mul(wq[:, :, 0], omy[:], omx[:])
                nc.vector.tensor_mul(wq[:, :, 1], dy[:], omx[:])
                nc.vector.tensor_mul(wq[:, :, 2], omy[:], dx[:])
                nc.vector.tensor_mul(wq[:, :, 3], dy[:], dx[:])

                # ---- stage G: gather + combine + transpose out ----
                tview = tabs[p][:]  # [HW,160] coef=160
                nbt = HW // POSB
                if half_split and p == npts - 1:
                    nbt = nbt // 2          # positions 0..8191 only
                dcp = desc_cols[p]
                for bt in range(nbt):
                    vt = sb.tile([128, GB, 320], F16, tag="vt")
                    for c0 in range(0, GB, dcp):
                        g0 = bt * GB + c0
                        if dcp == 1:
                            nc.gpsimd.indirect_dma_start(
                                out=vt[:, c0, :], out_offset=None, in_=tview,
                                in_offset=bass.IndirectOffsetOnAxis(
                                    ap=idx[:, g0:g0 + 1], axis=0))
                        else:
                            nc.gpsimd.indirect_dma_start(
                                out=vt[:, c0:c0 + dcp, :], out_offset=None, in_=tview,
                                in_offset=bass.IndirectOffsetOnAxis(
                                    ap=idx[:, g0:g0 + dcp], axis=0))
                    # weighted by quad weights: [128, GB*4, 80] * [128, GB*4, 1->80]
                    vt4 = bass.AP(vt.tensor, vt[:].offset,
                                  [vt[:].ap[0], [80, GB * 4], [1, 80]])
                    wslice = wq[:, bt * GB:(bt + 1) * GB, :]
                    wb = bass.AP(wq.tensor, wslice.offset,
                                 [wslice.ap[0], [1, GB * 4], [0, 80]])
                    nc.vector.tensor_tensor(out=vt4, in0=vt4, in1=wb, op=Alu.mult)
                    # sum x-pairs then y-pairs
                    nc.vector.tensor_add(vt[:, :, 0:160], vt[:, :, 0:160], vt[:, :, 160:320])
                    outc = sb.tile([128, GB, OFF], F16, tag="outc")
                    nc.vector.tensor_add(outc[:], vt[:, :, 0:80], vt[:, :, 80:160])

                    osb = sb.tile([OFF, POSB], F32, tag="osb")
                    for half in range(GB // 8):
                        ps2 = ppo.tile([OFF, 1024], F16, tag="ops")
                        for g in range(8):
                            nc.tensor.transpose(
                                ps2[:, g * 128:(g + 1) * 128],
                                outc[:, half * 8 + g, :], ident16[:])
                        nc.scalar.activation(osb[:, half * 1024:(half + 1) * 1024],
                                             ps2[:], Act.Identity, bias=bias_t[p][:])
                    nc.sync.dma_start(outb[p, :, bt * POSB:(bt + 1) * POSB], osb[:])
    nc.finalize()
    _nc_cache[key] = nc
    return nc


DESC_LIST = None                # diagnostic override: per-point desc_cols


def kernel(x, location, bias):
    from concourse import bass_utils
    x = np.ascontiguousarray(x, dtype=np.float32)
    location = np.ascontiguousarray(location, dtype=np.float32)
    bias = np.ascontiguousarray(bias, dtype=np.float32)
    nc = build_nc(desc_cols=DESC_LIST if DESC_LIST else DESC_COLS)

    xr = x.reshape(B, P, OFF, HW)
    lr = location.reshape(B, P, 2, HW)
    br = bias.reshape(P, OFF)
    HH = HW // 2
    sides = ([0, 1, 2, 3, 4], [5, 6, 7, 8, 4])
    in_maps = []
    for c in range(8):
        b, s = c // 2, c % 2
        pts = sides[s]
        lp = np.ascontiguousarray(lr[b, pts])
        if HALF_SPLIT and s == 1:
            # rotate point-4 locations so slots 0..8191 = true 8192..16383
            lp[NPTS - 1] = np.roll(lp[NPTS - 1], -HH, axis=1)
        in_maps.append(dict(xblk=np.ascontiguousarray(xr[b, pts]),
                            locp=lp,
                            biasb=np.ascontiguousarray(br[pts])))
    res = bass_utils.run_bass_kernel_spmd(nc, in_maps, list(range(8))).results
    out = np.empty((B, P, OFF, HW), np.float32)
    for c in range(8):
        b, s = c // 2, c % 2
        r = res[c]["outb"]
        if s == 0:
            out[b, 0:4] = r[0:4]
            out[b, 4, :, 0:HH] = r[4][:, 0:HH]
        else:
            out[b, 5:9] = r[0:4]
            if HALF_SPLIT:
                out[b, 4, :, HH:] = r[4][:, 0:HH]
            else:
                out[b, 4, :, HH:] = r[4][:, HH:]
    return out.reshape(B, C, H, W)


# revision 3
# speedup vs baseline: 1.2751x; 1.2751x over previous
"""nn_CollectConcat: bilinear deformable sampling, Trainium2 Bass kernel (v4).

v4 vs v3: gathers run through InstDMAGatherAnt (dma_gather) instead of
per-column indirect DMAs — one Pool op per `nidx` positions instead of one
per 128, eliminating the SWDGE fixed-cost bottleneck (~600us -> ~100us).

dma_gather needs int16 indices in a [16, N/16] pos%16-wrapped layout
replicated across the 8 gpsimd partition groups. That wrap is produced on
the PE: 8 matmuls with periodic selection matrices SEL_s0[c,p] =
(c == s0*16 + p%16) map idxf[c,pp] -> out[p,pp] = idx(pp*128 + s0*16 + p%16)
(replication included), then strided DVE casts interleave them into
Wrep[p, pp*8+s0] = idx(pos) for pos = (pp*8+s0)*16 + p%16.

Table rows pad to 512B ([x(i)|x(i+128)|pad]) so a single 1024B gather
element covers rows idx and idx+1 = all four bilinear corners (pads are
never read by the combine).
"""
import sys
sys.path.insert(0, '/opt/trn_rl_repo')
import numpy as np
import concourse.bass as bass
import concourse.bacc as bacc
import concourse.mybir as mybir
import concourse.tile as tile
import concourse.tile_sem_assignment as tsa
tsa.NUM_HWDGE_SEMS = 7  # 7 HW + 1 SW DMA sem domains: tail drain wait limit
from concourse.masks import make_identity

F32 = mybir.dt.float32
F16 = mybir.dt.float16
I32 = mybir.dt.int32
I16 = mybir.dt.int16
Alu = mybir.AluOpType
Act = mybir.ActivationFunctionType

B, C, H, W = 4, 720, 128, 128
P, OFF = 9, 80
HW = H * W                      # 16384
NPTS = 5                        # point-blocks per core
GB = 16                         # position-groups (of 128) per batch
POSB = GB * 128                 # positions per batch (2048)
QC = 4096                       # x-load quarter width
JQ = QC // 128                  # j iterations per quarter (32)
ROWE = 256                      # table row stride, fp16 elems (512B padded)
HALF_SPLIT = True               # point index 4: only positions 0..8191
GMODE = 1024                    # idxs per dma_gather op; 'ind' = v3 fallback

_nc_cache = {}


def build_nc(npts=NPTS, gmode=None, half_split=None):
    # gmode: int (num_idxs per dma_gather op), 'ind', or per-point list
    gmode = GMODE if gmode is None else gmode
    if not isinstance(gmode, (list, tuple)):
        gmode = [gmode] * npts
    gmode = tuple(gmode)
    half_split = HALF_SPLIT if half_split is None else half_split
    key = (npts, gmode, half_split)
    if key in _nc_cache:
        return _nc_cache[key]
    nc = bacc.Bacc("TRN2")
    xblk = nc.declare_dram_parameter("xblk", [npts, OFF, HW], F32, isOutput=False)
    locp = nc.declare_dram_parameter("locp", [npts, 2, HW], F32, isOutput=False)
    biasb = nc.declare_dram_parameter("biasb", [npts, OFF], F32, isOutput=False)
    outb = nc.declare_dram_parameter("outb", [npts, OFF, HW], F32, isOutput=True)
    tabs = [nc.dram_tensor(f"t2_{p}", [HW, ROWE], F16) for p in range(npts)]

    with tile.TileContext(nc) as tc:
        with tc.tile_pool(name="sbuf", bufs=2) as sb, \
             tc.tile_pool(name="cst", bufs=1) as cst, \
             tc.tile_pool(name="psc", bufs=1, space="PSUM") as ppc, \
             tc.tile_pool(name="pst", bufs=2, space="PSUM") as ppt, \
             tc.tile_pool(name="pso", bufs=2, space="PSUM") as ppo:
            ident = cst.tile([128, 128], F32)
            make_identity(nc, ident[:])
            ident16 = cst.tile([128, 128], F16, tag="id16", name="id16")
            nc.vector.tensor_copy(ident16[:], ident[:])
            warm = ppc.tile([128, 128], F32, tag="warm")
            nc.tensor.transpose(warm[:], ident[:], ident[:])  # PE sees ident

            # periodic selection bank: SEL[s0][c, p] = ident[c, s0*16 + p%16]
            need_wrap = any(g != 'ind' for g in gmode)
            sel = []
            if need_wrap:
                ipstr = ident[:].ap[0][0]
                for s0 in range(8):
                    st = cst.tile([128, 128], F32, tag=f"sel{s0}", name=f"sel{s0}")
                    src = bass.AP(ident.tensor, ident[:].offset + s0 * 16,
                                  [[ipstr, 128], [0, 8], [1, 16]])
                    dst = bass.AP(st.tensor, st[:].offset,
                                  [[st[:].ap[0][0], 128], [16, 8], [1, 16]])
                    nc.vector.tensor_copy(dst, src)
                    sel.append(st)

            bias_t = [cst.tile([OFF, 1], F32, tag=f"bias{p}", name=f"bias{p}") for p in range(npts)]
            for p in range(npts):
                nc.sync.dma_start(bias_t[p][:],
                                  bass.AP(biasb, p * OFF, [[1, OFF], [1, 1]]))

            for p in range(npts):
                gm = gmode[p]
                # ---- stage T: fp16 pair table, 512B rows [x(i)|x(i+128)|pad]
                for q in range(4):
                    img = sb.tile([OFF, QC + 128], F32, tag="img")
                    base = q * QC
                    ncols = QC + 128 if q < 3 else QC
                    nc.sync.dma_start(img[:, 0:ncols],
                                      xblk[p, :, base:base + ncols])
                    if q == 3:
                        nc.vector.memset(img[:, QC:QC + 128], 0.0)
                    outh = sb.tile([128, JQ * ROWE], F16, tag="outh")
                    istr = img[:].ap[0][0]
                    ostr = outh[:].ap[0][0]
                    pad = bass.AP(outh.tensor, outh[:].offset + 160,
                                  [[ostr, 128], [ROWE, JQ], [1, 96]])
                    nc.vector.memset(pad, 0.0)
                    j = 0
                    while j < JQ:
                        jn = min(3, JQ - j)
                        ps = ppt.tile([128, 480], F32, tag="tps")
                        for jj in range(jn):
                            in0 = bass.AP(img.tensor, j + jj, [[istr, OFF], [JQ, 128]])
                            in1 = bass.AP(img.tensor, j + jj + 128, [[istr, OFF], [JQ, 128]])
                            nc.tensor.transpose(ps[:, jj * 160:jj * 160 + 80],
                                                in0, ident[0:OFF, 0:OFF])
                            nc.tensor.transpose(ps[:, jj * 160 + 80:jj * 160 + 160],
                                                in1, ident[0:OFF, 0:OFF])
                        pin = bass.AP(ps.tensor, ps[:].offset,
                                      [[ps[:].ap[0][0], 128], [160, jn], [1, 160]])
                        pout = bass.AP(outh.tensor, outh[:].offset + j * ROWE,
                                       [[ostr, 128], [ROWE, jn], [1, 160]])
                        nc.scalar.activation(pout, pin, Act.Copy)
                        j += jn
                    # partition k -> rows base+JQ*k .. +JQ  (16KB runs, pads garbage)
                    nc.sync.dma_start(
                        bass.AP(tabs[p], base * ROWE, [[JQ * ROWE, 128], [1, JQ * ROWE]]),
                        outh[:])

                # ---- stage L: locations -> idx + weights ----
                locy_r = sb.tile([128, 128], F32, tag="locyr")
                locx_r = sb.tile([128, 128], F32, tag="locxr")
                nc.sync.dma_start(locy_r[:], bass.AP(locp, (2 * p) * HW, [[128, 128], [1, 128]]))
                nc.sync.dma_start(locx_r[:], bass.AP(locp, (2 * p + 1) * HW, [[128, 128], [1, 128]]))
                psy = ppc.tile([128, 128], F32, tag="locps")
                nc.tensor.transpose(psy[:], locy_r[:], ident[:])
                yq = sb.tile([128, 128], F32, tag="yq")
                nc.scalar.activation(yq[:], psy[:], Act.Copy)
                psx = ppc.tile([128, 128], F32, tag="locps")
                nc.tensor.transpose(psx[:], locx_r[:], ident[:])
                xq = sb.tile([128, 128], F32, tag="xq")
                nc.scalar.activation(xq[:], psx[:], Act.Copy)

                def floorf(q_, tag):
                    qi = sb.tile([128, 128], I32, tag=tag + "i")
                    nc.vector.tensor_copy(qi[:], q_[:])
                    qf = sb.tile([128, 128], F32, tag=tag + "f")
                    nc.vector.tensor_copy(qf[:], qi[:])
                    gt = sb.tile([128, 128], F32, tag=tag + "g")
                    nc.vector.tensor_tensor(out=gt[:], in0=qf[:], in1=q_[:], op=Alu.is_gt)
                    nc.vector.tensor_sub(qf[:], qf[:], gt[:])
                    d = sb.tile([128, 128], F32, tag=tag + "d")
                    nc.vector.tensor_sub(d[:], q_[:], qf[:])
                    return qf, d
                y0f, dy = floorf(yq, "y")
                x0f, dx = floorf(xq, "x")

                idxf = sb.tile([128, 128], F32, tag="idxf")
                nc.vector.scalar_tensor_tensor(out=idxf[:], in0=y0f[:], scalar=128.0,
                                               in1=x0f[:], op0=Alu.mult, op1=Alu.add)

                if gm == 'ind':
                    idx = sb.tile([128, 128], I32, tag="idx")
                    nc.vector.tensor_copy(idx[:], idxf[:])
                else:
                    # Wrep[p, pp*8+s0] = idx((pp*8+s0)*16 + p%16), int16,
                    # replicated across partition groups of 16.
                    wrep = sb.tile([128, 1024], I16, tag="wrep")
                    wstr = wrep[:].ap[0][0]
                    for s0 in range(8):
                        wps = ppc.tile([128, 128], F32, tag="wps")
                        nc.tensor.matmul(wps[:], sel[s0][:], idxf[:])
                        dst = bass.AP(wrep.tensor, wrep[:].offset + s0,
                                      [[wstr, 128], [8, 128]])
                        nc.vector.tensor_copy(dst, wps[:])

                omy = sb.tile([128, 128], F32, tag="omy")
                nc.vector.tensor_scalar(out=omy[:], in0=dy[:], scalar1=-1.0,
                                        scalar2=1.0, op0=Alu.mult, op1=Alu.add)
                omx = sb.tile([128, 128], F32, tag="omx")
                nc.vector.tensor_scalar(out=omx[:], in0=dx[:], scalar1=-1.0,
                                        scalar2=1.0, op0=Alu.mult, op1=Alu.add)
                # pair weights, fp16: wqA = (w00, w10) for row idx,
                # wqB = (w01, w11) for row idx+1
                wqA = sb.tile([128, 128, 2], F16, tag="wqA")
                nc.vector.tensor_mul(wqA[:, :, 0], omy[:], omx[:])
                nc.vector.tensor_mul(wqA[:, :, 1], dy[:], omx[:])
                wqB = sb.tile([128, 128, 2], F16, tag="wqB")
                nc.vector.tensor_mul(wqB[:, :, 0], omy[:], dx[:])
                nc.vector.tensor_mul(wqB[:, :, 1], dy[:], dx[:])

                # ---- stage G ----
                nbt = HW // POSB
                if half_split and p == npts - 1:
                    nbt = nbt // 2          # positions 0..8191 only
                for bt in range(nbt):
                    vt = sb.tile([128, GB, 2 * ROWE], F16, tag="vt")
                    if gm == 'ind':
                        # coef (indirect row stride) = trailing size = ROWE;
                        # descriptor length = out run = 2*ROWE elems
                        tview = bass.AP(tabs[p], 0, [[ROWE, HW], [1, ROWE]])
                        for c0 in range(GB):
                            nc.gpsimd.indirect_dma_start(
                                out=vt[:, c0, :], out_offset=None, in_=tview,
                                in_offset=bass.IndirectOffsetOnAxis(
                                    ap=idx[:, bt * GB + c0:bt * GB + c0 + 1],
                                    axis=0))
                    else:
                        gcols = gm // 16        # Wrep columns per op
                        gver = gm // 128        # vt groups per op
                        # HW-1 rows: row i's 512-elem (2-row) read must stay
                        # inside the [HW, ROWE] buffer; idx <= 16254 anyway
                        tv = bass.AP(tabs[p], 0, [[ROWE, HW - 1], [1, 2 * ROWE]])
                        for k in range(GB // gver):
                            s_lo = bt * (GB * 8) + k * gcols
                            nc.gpsimd.dma_gather(
                                out_ap=vt[:, k * gver:(k + 1) * gver, :],
                                in_ap=tv,
                                idxs_ap=wrep[:, s_lo:s_lo + gcols],
                                num_idxs=gm,
                                num_idxs_reg=gm,
                                elem_size=2 * ROWE,
                                elem_step=ROWE)
                    # weighted corners; pads [160:256],[416:512] never read.
                    # four 3D multiplies: corner block x broadcast weight lane
                    for woff, (wt, lane) in zip(
                            (0, 80, ROWE, ROWE + 80),
                            ((wqA, 0), (wqA, 1), (wqB, 0), (wqB, 1))):
                        vc = bass.AP(vt.tensor, vt[:].offset + woff,
                                     [vt[:].ap[0], [2 * ROWE, GB], [1, 80]])
                        wsl = wt[:, bt * GB:(bt + 1) * GB, lane]
                        wc = bass.AP(wt.tensor, wsl.offset,
                                     [wt[:].ap[0], [2, GB], [0, 80]])
                        nc.vector.tensor_tensor(out=vc, in0=vc, in1=wc, op=Alu.mult)
                    # sum rows: [0:160] += [256:416]
                    a0 = bass.AP(vt.tensor, vt[:].offset,
                                 [vt[:].ap[0], [2 * ROWE, GB], [1, 160]])
                    a1 = bass.AP(vt.tensor, vt[:].offset + ROWE,
                                 [vt[:].ap[0], [2 * ROWE, GB], [1, 160]])
                    nc.vector.tensor_tensor(out=a0, in0=a0, in1=a1, op=Alu.add)
                    outc = sb.tile([128, GB, OFF], F16, tag="outc")
                    b0 = bass.AP(vt.tensor, vt[:].offset,
                                 [vt[:].ap[0], [2 * ROWE, GB], [1, 80]])
                    b1 = bass.AP(vt.tensor, vt[:].offset + 80,
                                 [vt[:].ap[0], [2 * ROWE, GB], [1, 80]])
                    nc.vector.tensor_tensor(out=outc[:], in0=b0, in1=b1, op=Alu.add)

                    osb = sb.tile([OFF, POSB], F32, tag="osb")
                    for half in range(GB // 8):
                        ps2 = ppo.tile([OFF, 1024], F16, tag="ops")
                        for g in range(8):
                            nc.tensor.transpose(
                                ps2[:, g * 128:(g + 1) * 128],
                                outc[:, half * 8 + g, :], ident16[:])
                        nc.scalar.activation(osb[:, half * 1024:(half + 1) * 1024],
                                             ps2[:], Act.Identity, bias=bias_t[p][:])
                    nc.sync.dma_start(outb[p, :, bt * POSB:(bt + 1) * POSB], osb[:])
    nc.finalize()
    _nc_cache[key] = nc
    return nc


GLIST = None                    # diagnostic override: per-point gmode


def kernel(x, location, bias):
    from concourse import bass_utils
    x = np.ascontiguousarray(x, dtype=np.float32)
    location = np.ascontiguousarray(location, dtype=np.float32)
    bias = np.ascontiguousarray(bias, dtype=np.float32)
    nc = build_nc(gmode=GLIST if GLIST else GMODE)

    xr = x.reshape(B, P, OFF, HW)
    lr = location.reshape(B, P, 2, HW)
    br = bias.reshape(P, OFF)
    HH = HW // 2
    sides = ([0, 1, 2, 3, 4], [5, 6, 7, 8, 4])
    in_maps = []
    for c in range(8):
        b, s = c // 2, c % 2
        pts = sides[s]
        lp = np.ascontiguousarray(lr[b, pts])
        if HALF_SPLIT and s == 1:
            # rotate point-4 locations so slots 0..8191 = true 8192..16383
            lp[NPTS - 1] = np.roll(lp[NPTS - 1], -HH, axis=1)
        in_maps.append(dict(xblk=np.ascontiguousarray(xr[b, pts]),
                            locp=lp,
                            biasb=np.ascontiguousarray(br[pts])))
    res = bass_utils.run_bass_kernel_spmd(nc, in_maps, list(range(8))).results
    out = np.empty((B, P, OFF, HW), np.float32)
    for c in range(8):
        b, s = c // 2, c % 2
        r = res[c]["outb"]
        if s == 0:
            out[b, 0:4] = r[0:4]
            out[b, 4, :, 0:HH] = r[4][:, 0:HH]
        else:
            out[b, 5:9] = r[0:4]
            out[b, 4, :, HH:] = r[4][:, 0:HH]
    return out.reshape(B, C, H, W)
